# revision 9
# baseline (speedup 1.0000x reference)
"""AOSPredictionLayer — Trainium2 Bass kernel (8 NeuronCores, expert-sharded).

Problem: pred[b,n] = <ui_mlp(concat(u,i))[b], relation_mlp[s[b,n]](concat(a,o)[b,n])>
with B=512, N=32, R=8 relations, MLP dims 512->512->256->128 (leaky_relu 0.01).

MoE routing: host groups the B*N = 16384 tokens by relation id; core e gets
exactly the tokens of relation e (per-relation weights sharded, UI MLP weights
replicated). On device (bf16 matmul inputs, fp32 PSUM accumulate):
  - shared UI MLP over all 512 batch rows (feature-major), transposed to
    token-major via 4 PE transposes into one PSUM bank + one wide DVE copy
  - the expert 3-layer MLP over its tokens in <=512-col chunks (feature-major,
    bias+leaky_relu fused on the Scalar engine)
  - one-hot gather of ui_emb[b(t)] per token (fp16 row-id compare on DVE
    tensor_scalar, 0/1 exact in bf16; gather by matmul)
  - dot product via elementwise multiply + ones-matmul partition reduce;
    pred stored straight from PSUM
All tensors reach SBUF via few large DMAs (one per tensor / token chunk,
[128, k, n] layouts) spread across the SP/Pool/Act queues; everything is bf16
on the wire except biases (f32) and row ids (fp16), halving the serialized
DMA-pipe time versus f32.
"""
import sys

for _p in ("/opt/trn_rl_repo", "/opt/pypackages"):
    if _p not in sys.path:
        sys.path.append(_p)

import numpy as np
import ml_dtypes

import concourse.tile as tile
from concourse import bacc, mybir
from concourse.masks import make_identity
from concourse.bass_utils import run_bass_kernel_spmd

F32 = mybir.dt.float32
BF16 = mybir.dt.bfloat16
F16 = mybir.dt.float16

BF16_NP = ml_dtypes.bfloat16

B = 512            # batch rows
N_TOK = 32         # tokens per batch row
IN1 = 256          # a/o embedding dim
IN2 = 256          # u/i embedding dim
HID = [512, 256, 128]
R = 8              # relations == cores
N_CORES = 8

D_AO = 2 * IN1
D_UI = 2 * IN2
DIMS = [D_AO] + HID          # [512, 512, 256, 128]
KC = [d // 128 for d in DIMS]            # k-chunks per layer input: [4,4,2,1]
MC = [d // 128 for d in DIMS[1:]]        # m-chunks per layer output: [4,2,1]
BOFF = [0, 4, 6]             # bias column offset per layer in the [128,7] pack


def _chunks(tp):
    """Split tp columns into <=512 chunks, near-equal, plus a small tail
    chunk (~128) so the final dot->store chain is short."""
    tail = 128 if tp > 640 else 0
    body = tp - tail
    nch = (body + 511) // 512
    base, rem = divmod(body, nch)
    sizes = [base + 1] * rem + [base] * (nch - rem)
    if tail:
        sizes.append(tail)
    out, s = [], 0
    for n in sizes:
        out.append((s, n))
        s += n
    return out


def _build_kernel(tp, chunk_cs):
    """Per-core Bacc graph for TP=tp padded tokens (identical on all cores;
    per-core data arrives via in_maps)."""
    nc = bacc.Bacc("TRN2", target_bir_lowering=False, debug=False,
                   num_devices=N_CORES)

    xT_d = nc.dram_tensor("xT", [D_AO, tp], BF16, kind="ExternalInput").ap()
    w_d = [nc.dram_tensor(f"w{i+1}", [DIMS[i], DIMS[i+1]], BF16,
                          kind="ExternalInput").ap() for i in range(3)]
    wu_d = [nc.dram_tensor(f"wu{i+1}", [DIMS[i], DIMS[i+1]], BF16,
                           kind="ExternalInput").ap() for i in range(3)]
    bp_d = nc.dram_tensor("bpack", [128, 7], F32, kind="ExternalInput").ap()
    bup_d = nc.dram_tensor("bupack", [128, 7], F32, kind="ExternalInput").ap()
    uiT_d = nc.dram_tensor("uiT", [D_UI, B], BF16, kind="ExternalInput").ap()
    bids_d = nc.dram_tensor("bids", [128, tp], F16, kind="ExternalInput").ap()
    iota_d = nc.dram_tensor("iota4", [128, 4], F32, kind="ExternalInput").ap()
    pred_d = nc.dram_tensor("pred", [1, tp], F32, kind="ExternalOutput").ap()

    BC = B // 128
    chunks = _chunks(tp)

    with tile.TileContext(nc) as tc:
        with (
            tc.tile_pool(name="wts", bufs=1) as wts,
            tc.tile_pool(name="ui", bufs=1) as uip,
            tc.tile_pool(name="xin", bufs=len(chunks)) as xin,
            tc.tile_pool(name="act", bufs=2) as actp,
            tc.tile_pool(name="mmps", bufs=7, space="PSUM") as mmps,
            tc.tile_pool(name="dotps", bufs=1, space="PSUM") as dotps,
        ):
            # ---- DMAs: few and large, spread over the Act/SP/Pool queues ----
            # Act queue: UI weights (first UI matmul gates on wu1 halves),
            # then the LUT warm-up so the Lrelu table loads during the waits.
            wu1_h = []
            for h in range(2):
                t = wts.tile([128, 2, DIMS[1]], BF16, tag=f"wu1h{h}")
                nc.scalar.dma_start(
                    t[:], wu_d[0].rearrange("(k p) m -> p k m", p=128)[:, 2*h:2*h+2, :])
                wu1_h.append(t)
            lut_in = uip.tile([1, 1], F32, tag="lutin")
            nc.vector.memset(lut_in[:], 0.0)
            lut_out = uip.tile([1, 1], F32, tag="lutout")
            nc.scalar.activation(lut_out[:], lut_in[:],
                                 mybir.ActivationFunctionType.Lrelu,
                                 bias=0.0, scale=1.0, alpha=0.01)
            bu_s = wts.tile([128, 7], F32, tag="bu")
            nc.scalar.dma_start(bu_s[:], bup_d[:])

            # SP queue: UI input halves, then token chunks.
            uiT_h = []
            for h in range(2):
                t = wts.tile([128, 2, B], BF16, tag=f"uiTh{h}")
                nc.sync.dma_start(
                    t[:], uiT_d.rearrange("(k p) c -> p k c", p=128)[:, 2*h:2*h+2, :])
                uiT_h.append(t)
            wu23_s = []
            for i in (1, 2):
                t = wts.tile([128, KC[i], DIMS[i+1]], BF16, tag=f"wu{i+1}")
                nc.sync.dma_start(
                    t[:], wu_d[i].rearrange("(k p) m -> p k m", p=128))
                wu23_s.append(t)
            xcs = []
            xT_r = xT_d.rearrange("(k p) c -> p k c", p=128)
            for ci, (s0, n) in enumerate(chunks):
                t = xin.tile([128, 4, n], BF16, tag="x", name=f"x{ci}")
                nc.sync.dma_start(t[:], xT_r[:, :, s0:s0 + n])
                xcs.append(t)

            # Pool queue (SWDGE): expert weights, biases, row ids, identity.
            b_s = wts.tile([128, 7], F32, tag="b")
            nc.gpsimd.dma_start(b_s[:], bp_d[:])
            iota_s = wts.tile([128, 4], F32, tag="iota")
            nc.gpsimd.dma_start(iota_s[:], iota_d[:])
            w_s = []
            for i in range(3):
                t = wts.tile([128, KC[i], DIMS[i+1]], BF16, tag=f"w{i+1}")
                nc.gpsimd.dma_start(
                    t[:], w_d[i].rearrange("(k p) m -> p k m", p=128))
                w_s.append(t)
            bids_s = wts.tile([128, tp], F16, tag="bids")
            nc.gpsimd.dma_start(bids_s[:], bids_d[:])

            ident = uip.tile([128, 128], F32, tag="ident")
            make_identity(nc, ident[:])
            ones = uip.tile([128, 1], BF16, tag="ones")
            nc.vector.memset(ones[:], 1.0)

            def mlp_layer(li, in_of_k, out_t, ws, bs, n_cols, order=None):
                """Feature-major layer li: out[:,m,:] = lrelu(sum_k w.T@in + b).
                in_of_k(k) -> [128, n_cols] AP; out_t [128, MC[li], n_cols].
                order: explicit (m, k) pass emission order (default m-major)."""
                kc, mc = KC[li], MC[li]
                if order is None:
                    order = [(m, k) for m in range(mc) for k in range(kc)]
                pss = [None] * mc
                seen = [0] * mc
                for m, k in order:
                    if pss[m] is None:
                        pss[m] = mmps.tile([128, 512], F32, tag="mm",
                                           name=f"mm_l{li}m{m}")
                    seen[m] += 1
                    nc.tensor.matmul(
                        pss[m][:, :n_cols], ws(k)[:, m * 128:(m + 1) * 128],
                        in_of_k(k), start=(seen[m] == 1), stop=(seen[m] == kc))
                    if seen[m] == kc:
                        nc.scalar.activation(
                            out_t[:, m, :], pss[m][:, :n_cols],
                            mybir.ActivationFunctionType.Lrelu,
                            bias=bs[:, BOFF[li] + m:BOFF[li] + m + 1],
                            scale=1.0, alpha=0.01)

            def chunk_mlp(ci, n):
                h1 = actp.tile([128, 4, n], BF16, tag="h1", name=f"h1c{ci}")
                mlp_layer(0, lambda k: xcs[ci][:, k, :], h1,
                          lambda k: w_s[0][:, k, :], b_s, n)
                h2 = actp.tile([128, 2, n], BF16, tag="h2", name=f"h2c{ci}")
                mlp_layer(1, lambda k: h1[:, k, :], h2,
                          lambda k: w_s[1][:, k, :], b_s, n)
                h3 = actp.tile([128, 1, n], BF16, tag="h3", name=f"h3c{ci}")
                mlp_layer(2, lambda k: h2[:, k, :], h3,
                          lambda k: w_s[2][:, k, :], b_s, n)
                return h3

            # ---- UI MLP over all B rows (feature-major), interleaved with
            # chunk 0 so the PE queue never starves while weights stream ----
            # L1 passes k-ordered: the k0/k1 halves of uiT/wu1 land first.
            ui1 = uip.tile([128, 4, B], BF16, tag="ui1")
            l1_order = ([(m, k) for k in (0, 1) for m in range(4)] +
                        [(m, k) for m in range(4) for k in (2, 3)])
            mlp_layer(0, lambda k: uiT_h[k // 2][:, k % 2, :], ui1,
                      lambda k: wu1_h[k // 2][:, k % 2, :], bu_s, B,
                      order=l1_order)
            ui2 = uip.tile([128, 2, B], BF16, tag="ui2")
            mlp_layer(1, lambda k: ui1[:, k, :], ui2,
                      lambda k: wu23_s[0][:, k, :], bu_s, B)
            h3_c0 = chunk_mlp(0, chunks[0][1])
            ui3 = uip.tile([128, 1, B], F32, tag="ui3")
            mlp_layer(2, lambda k: ui2[:, k, :], ui3,
                      lambda k: wu23_s[1][:, k, :], bu_s, B)

            # transpose ui3 [128d x B] -> token-major [128b x BC x 128d]:
            # 4 PE transposes into one PSUM bank, one wide DVE copy out.
            tps = mmps.tile([128, 512], F32, tag="mm")
            for c in range(BC):
                nc.tensor.transpose(tps[:, c * 128:(c + 1) * 128],
                                    ui3[:, 0, c * 128:(c + 1) * 128], ident[:])
            ui3_tok = uip.tile([128, BC, 128], BF16, tag="ui3tok")
            nc.vector.tensor_copy(ui3_tok[:], tps[:])

            # ---- token chunks ----
            for ci, (s0, n) in enumerate(chunks):
                h3 = h3_c0 if ci == 0 else chunk_mlp(ci, n)

                cs = chunk_cs[ci]
                # one-hot[b, t] = (b == bids[t]) for the b-chunks present
                oh = actp.tile([128, BC, n], BF16, tag="oh")
                for c in cs:
                    nc.vector.tensor_scalar(
                        out=oh[:, c, :], in0=bids_s[:, s0:s0 + n],
                        scalar1=iota_s[:, c:c + 1], scalar2=None,
                        op0=mybir.AluOpType.is_equal)
                # gathered ui columns: uig = ui3_tok.T @ oh  (exact selection)
                psg = mmps.tile([128, 512], F32, tag="mm")
                for j, c in enumerate(cs):
                    nc.tensor.matmul(psg[:, :n], ui3_tok[:, c, :], oh[:, c, :],
                                     start=(j == 0), stop=(j == len(cs) - 1))
                # pred = ones.T @ (h3 * uig)   (partition reduce over d=128)
                prod = actp.tile([128, n], BF16, tag="prod")
                nc.vector.tensor_tensor(out=prod[:], in0=h3[:, 0, :],
                                        in1=psg[:, :n],
                                        op=mybir.AluOpType.mult)
                psd = dotps.tile([1, 512], F32, tag="dot")
                nc.tensor.matmul(psd[:, :n], ones[:], prod[:],
                                 start=True, stop=True)
                pc = actp.tile([1, n], F32, tag="predc")
                nc.vector.tensor_copy(pc[:], psd[:, :n])
                nc.sync.dma_start(pred_d[:, s0:s0 + n], pc[:])

    nc.compile()
    return nc


def _prepare(u_emb, i_emb, a_emb, o_emb, s):
    """Host-side sharding: route tokens to cores by relation id."""
    s_flat = np.asarray(s).reshape(-1).astype(np.int64)
    n_tokens = s_flat.shape[0]
    X = np.concatenate(
        [np.asarray(a_emb, dtype=np.float32).reshape(n_tokens, IN1),
         np.asarray(o_emb, dtype=np.float32).reshape(n_tokens, IN1)],
        axis=1).astype(BF16_NP)
    uiT = np.ascontiguousarray(
        np.concatenate([np.asarray(u_emb, dtype=np.float32),
                        np.asarray(i_emb, dtype=np.float32)],
                       axis=1).astype(BF16_NP).T)

    idx = [np.flatnonzero(s_flat == e) for e in range(R)]
    tp = max(256, -(-max(max(len(ix) for ix in idx), 1) // 4) * 4)

    iota4 = np.ascontiguousarray(
        (np.arange(128, dtype=np.float32)[:, None] +
         128.0 * np.arange(4, dtype=np.float32)[None, :]))

    in_maps = []
    chunks = _chunks(tp)
    chunk_cs = [set() for _ in chunks]
    for e in range(R):
        # sort tokens by batch row within the expert: narrows per-chunk b-range
        order = np.argsort(idx[e] // N_TOK, kind="stable")
        idx[e] = idx[e][order]
        ix = idx[e]
        pad = np.full(tp, n_tokens - 1, dtype=np.int64)
        pad[:len(ix)] = ix
        xT = np.ascontiguousarray(X[pad].T)
        b_of_tok = pad // N_TOK
        bids = np.ascontiguousarray(np.broadcast_to(
            b_of_tok.astype(np.float16)[None, :], (128, tp)))
        cg = b_of_tok // 128
        for ci, (s0, n) in enumerate(chunks):
            for c in np.unique(cg[s0:s0 + n]):
                chunk_cs[ci].add(int(c))
        in_maps.append({"xT": xT, "uiT": uiT, "bids": bids, "iota4": iota4})
    chunk_cs = [sorted(cset) for cset in chunk_cs]
    return in_maps, idx, tp, chunk_cs


def _pack_bias(bs):
    """Stack per-layer bias vectors into a [128, n_cols] f32 pack."""
    cols = []
    for b in bs:
        cols.append(np.asarray(b, dtype=np.float32).reshape(-1, 128).T)
    return np.ascontiguousarray(np.concatenate(cols, axis=1))


def kernel(u_emb, i_emb, a_emb, o_emb, s,
           W1, b1, W2, b2, W3, b3,
           Wu1, bu1, Wu2, bu2, Wu3, bu3):
    in_maps, idx, tp, chunk_cs = _prepare(u_emb, i_emb, a_emb, o_emb, s)
    ws = {"W1": W1, "W2": W2, "W3": W3, "Wu1": Wu1, "Wu2": Wu2, "Wu3": Wu3}
    bupack = _pack_bias([bu1, bu2, bu3])
    for e in range(R):
        m = in_maps[e]
        for li in range(3):
            m[f"w{li+1}"] = np.ascontiguousarray(
                np.asarray(ws[f"W{li+1}"][e], dtype=np.float32).astype(BF16_NP))
            m[f"wu{li+1}"] = np.ascontiguousarray(
                np.asarray(ws[f"Wu{li+1}"], dtype=np.float32).astype(BF16_NP))
        m["bpack"] = _pack_bias([b1[e], b2[e], b3[e]])
        m["bupack"] = bupack

    nc = _build_kernel(tp, chunk_cs)
    res = run_bass_kernel_spmd(nc, in_maps, core_ids=list(range(N_CORES)))

    s_arr = np.asarray(s)
    out = np.zeros(s_arr.size, dtype=np.float32)
    for e in range(R):
        pred = res.results[e]["pred"].reshape(-1)
        out[idx[e]] = pred[:len(idx[e])]
    return out.reshape(s_arr.shape)


# revision 10
# speedup vs baseline: 1.0388x; 1.0388x over previous
"""AOSPredictionLayer — Trainium2 Bass kernel (8 NeuronCores, expert-sharded).

Problem: pred[b,n] = <ui_mlp(concat(u,i))[b], relation_mlp[s[b,n]](concat(a,o)[b,n])>
with B=512, N=32, R=8 relations, MLP dims 512->512->256->128 (leaky_relu 0.01).

MoE routing: host groups the B*N = 16384 tokens by relation id; core e gets
exactly the tokens of relation e (per-relation weights sharded, UI MLP weights
replicated). On device (bf16 matmul inputs, fp32 PSUM accumulate):
  - shared UI MLP over all 512 batch rows (feature-major), transposed to
    token-major via 4 PE transposes into one PSUM bank + one wide DVE copy
  - the expert 3-layer MLP over its tokens in <=512-col chunks (feature-major,
    bias+leaky_relu fused on the Scalar engine)
  - one-hot gather of ui_emb[b(t)] per token (fp16 row-id compare on DVE
    tensor_scalar, 0/1 exact in bf16; gather by matmul)
  - dot product via elementwise multiply + ones-matmul partition reduce;
    pred stored straight from PSUM
All tensors reach SBUF via few large DMAs (one per tensor / token chunk,
[128, k, n] layouts) spread across the SP/Pool/Act queues; everything is bf16
on the wire except biases (f32) and row ids (fp16), halving the serialized
DMA-pipe time versus f32.
"""
import sys

for _p in ("/opt/trn_rl_repo", "/opt/pypackages"):
    if _p not in sys.path:
        sys.path.append(_p)

import numpy as np
import ml_dtypes

import concourse.tile as tile
from concourse import bacc, mybir
from concourse.masks import make_identity
from concourse.bass_utils import run_bass_kernel_spmd

F32 = mybir.dt.float32
BF16 = mybir.dt.bfloat16
F16 = mybir.dt.float16

BF16_NP = ml_dtypes.bfloat16

B = 512            # batch rows
N_TOK = 32         # tokens per batch row
IN1 = 256          # a/o embedding dim
IN2 = 256          # u/i embedding dim
HID = [512, 256, 128]
R = 8              # relations == cores
N_CORES = 8

D_AO = 2 * IN1
D_UI = 2 * IN2
DIMS = [D_AO] + HID          # [512, 512, 256, 128]
KC = [d // 128 for d in DIMS]            # k-chunks per layer input: [4,4,2,1]
MC = [d // 128 for d in DIMS[1:]]        # m-chunks per layer output: [4,2,1]
BOFF = [0, 4, 6]             # bias column offset per layer in the [128,7] pack


def _chunks(tp):
    """Split tp columns into <=512 chunks, near-equal, plus a small tail
    chunk (~128) so the final dot->store chain is short."""
    tail = 128 if tp > 640 else 0
    body = tp - tail
    nch = (body + 511) // 512
    base, rem = divmod(body, nch)
    sizes = [base + 1] * rem + [base] * (nch - rem)
    if tail:
        sizes.append(tail)
    out, s = [], 0
    for n in sizes:
        out.append((s, n))
        s += n
    return out


def _build_kernel(tp, chunk_cs):
    """Per-core Bacc graph for TP=tp padded tokens (identical on all cores;
    per-core data arrives via in_maps)."""
    nc = bacc.Bacc("TRN2", target_bir_lowering=False, debug=False,
                   num_devices=N_CORES)

    xT_d = nc.dram_tensor("xT", [D_AO, tp], BF16, kind="ExternalInput").ap()
    w_d = [nc.dram_tensor(f"w{i+1}", [DIMS[i], DIMS[i+1]], BF16,
                          kind="ExternalInput").ap() for i in range(3)]
    wu_d = [nc.dram_tensor(f"wu{i+1}", [DIMS[i], DIMS[i+1]], BF16,
                           kind="ExternalInput").ap() for i in range(3)]
    bp_d = nc.dram_tensor("bpack", [128, 7], F32, kind="ExternalInput").ap()
    bup_d = nc.dram_tensor("bupack", [128, 7], F32, kind="ExternalInput").ap()
    uiT_d = nc.dram_tensor("uiT", [D_UI, B], BF16, kind="ExternalInput").ap()
    bids_d = nc.dram_tensor("bids", [128, tp], F16, kind="ExternalInput").ap()
    iota_d = nc.dram_tensor("iota4", [128, 4], F32, kind="ExternalInput").ap()
    pred_d = nc.dram_tensor("pred", [1, tp], F32, kind="ExternalOutput").ap()

    BC = B // 128
    chunks = _chunks(tp)

    with tile.TileContext(nc) as tc:
        with (
            tc.tile_pool(name="wts", bufs=1) as wts,
            tc.tile_pool(name="ui", bufs=1) as uip,
            tc.tile_pool(name="xin", bufs=len(chunks)) as xin,
            tc.tile_pool(name="act", bufs=2) as actp,
            tc.tile_pool(name="mmps", bufs=7, space="PSUM") as mmps,
            tc.tile_pool(name="dotps", bufs=1, space="PSUM") as dotps,
        ):
            # ---- PE warm-up: pe_busy_start is pinned at the first PE busy
            # moment and the clock reaches 2.4GHz 3us later, so keep the PE
            # grinding on throwaway matmuls while the first DMAs land ----
            ones = uip.tile([128, 1], BF16, tag="ones")
            nc.vector.memset(ones[:], 1.0)
            dum_in = uip.tile([128, 512], BF16, tag="dumin")
            nc.vector.memset(dum_in[:], 0.0)
            psdum = dotps.tile([1, 512], F32, tag="dot", name="psdum")
            for _ in range(7):
                nc.tensor.matmul(psdum[:], ones[:], dum_in[:],
                                 start=True, stop=True)

            # ---- DMAs: few and large, spread over the Act/SP/Pool queues ----
            # Act queue: UI weights (first UI matmul gates on wu1 halves),
            # then the LUT warm-up so the Lrelu table loads during the waits.
            wu1_h = []
            for h in range(2):
                t = wts.tile([128, 2, DIMS[1]], BF16, tag=f"wu1h{h}")
                nc.scalar.dma_start(
                    t[:], wu_d[0].rearrange("(k p) m -> p k m", p=128)[:, 2*h:2*h+2, :])
                wu1_h.append(t)
            lut_in = uip.tile([1, 1], F32, tag="lutin")
            nc.vector.memset(lut_in[:], 0.0)
            lut_out = uip.tile([1, 1], F32, tag="lutout")
            nc.scalar.activation(lut_out[:], lut_in[:],
                                 mybir.ActivationFunctionType.Lrelu,
                                 bias=0.0, scale=1.0, alpha=0.01)
            bu_s = wts.tile([128, 7], F32, tag="bu")
            nc.scalar.dma_start(bu_s[:], bup_d[:])

            # SP queue: UI input halves, then token chunks.
            uiT_h = []
            for h in range(2):
                t = wts.tile([128, 2, B], BF16, tag=f"uiTh{h}")
                nc.sync.dma_start(
                    t[:], uiT_d.rearrange("(k p) c -> p k c", p=128)[:, 2*h:2*h+2, :])
                uiT_h.append(t)
            wu23_s = []
            for i in (1, 2):
                t = wts.tile([128, KC[i], DIMS[i+1]], BF16, tag=f"wu{i+1}")
                nc.sync.dma_start(
                    t[:], wu_d[i].rearrange("(k p) m -> p k m", p=128))
                wu23_s.append(t)
            xcs = []
            xT_r = xT_d.rearrange("(k p) c -> p k c", p=128)
            for ci, (s0, n) in enumerate(chunks):
                t = xin.tile([128, 4, n], BF16, tag="x", name=f"x{ci}")
                nc.sync.dma_start(t[:], xT_r[:, :, s0:s0 + n])
                xcs.append(t)

            # Pool queue (SWDGE): expert weights, biases, row ids, identity.
            b_s = wts.tile([128, 7], F32, tag="b")
            nc.gpsimd.dma_start(b_s[:], bp_d[:])
            iota_s = wts.tile([128, 4], F32, tag="iota")
            nc.gpsimd.dma_start(iota_s[:], iota_d[:])
            w_s = []
            for i in range(3):
                t = wts.tile([128, KC[i], DIMS[i+1]], BF16, tag=f"w{i+1}")
                nc.gpsimd.dma_start(
                    t[:], w_d[i].rearrange("(k p) m -> p k m", p=128))
                w_s.append(t)
            bids_s = wts.tile([128, tp], F16, tag="bids")
            nc.gpsimd.dma_start(bids_s[:], bids_d[:])

            ident = uip.tile([128, 128], F32, tag="ident")
            make_identity(nc, ident[:])

            def mlp_layer(li, in_of_k, out_t, ws, bs, n_cols, order=None):
                """Feature-major layer li: out[:,m,:] = lrelu(sum_k w.T@in + b).
                in_of_k(k) -> [128, n_cols] AP; out_t [128, MC[li], n_cols].
                order: explicit (m, k) pass emission order (default m-major)."""
                kc, mc = KC[li], MC[li]
                if order is None:
                    order = [(m, k) for m in range(mc) for k in range(kc)]
                pss = [None] * mc
                seen = [0] * mc
                for m, k in order:
                    if pss[m] is None:
                        pss[m] = mmps.tile([128, 512], F32, tag="mm",
                                           name=f"mm_l{li}m{m}")
                    seen[m] += 1
                    nc.tensor.matmul(
                        pss[m][:, :n_cols], ws(k)[:, m * 128:(m + 1) * 128],
                        in_of_k(k), start=(seen[m] == 1), stop=(seen[m] == kc))
                    if seen[m] == kc:
                        nc.scalar.activation(
                            out_t[:, m, :], pss[m][:, :n_cols],
                            mybir.ActivationFunctionType.Lrelu,
                            bias=bs[:, BOFF[li] + m:BOFF[li] + m + 1],
                            scale=1.0, alpha=0.01)

            def chunk_mlp(ci, n, h1=None):
                if h1 is None:
                    h1 = actp.tile([128, 4, n], BF16, tag="h1",
                                   name=f"h1c{ci}")
                    mlp_layer(0, lambda k: xcs[ci][:, k, :], h1,
                              lambda k: w_s[0][:, k, :], b_s, n)
                h2 = actp.tile([128, 2, n], BF16, tag="h2", name=f"h2c{ci}")
                mlp_layer(1, lambda k: h1[:, k, :], h2,
                          lambda k: w_s[1][:, k, :], b_s, n,
                          order=[(m, k) for k in range(4) for m in range(2)])
                h3 = actp.tile([128, 1, n], BF16, tag="h3", name=f"h3c{ci}")
                mlp_layer(2, lambda k: h2[:, k, :], h3,
                          lambda k: w_s[2][:, k, :], b_s, n,
                          order=[(0, k) for k in range(2)])
                return h3

            # ---- UI MLP over all B rows (feature-major) ----
            # L1: the k0/k1 halves of uiT/wu1 land first; finish each m's
            # k0/k1 passes before the k2/k3 half arrives.
            ui1 = uip.tile([128, 4, B], BF16, tag="ui1")
            l1_order = [(m, k) for kk in (0, 2) for m in range(4)
                        for k in (kk, kk + 1)]
            mlp_layer(0, lambda k: uiT_h[k // 2][:, k % 2, :], ui1,
                      lambda k: wu1_h[k // 2][:, k % 2, :], bu_s, B,
                      order=l1_order)
            # L2/L3 k-major: k inputs are the L1/L2 acts, which retire in
            # m order on the Act queue — k-major consumes them just-in-time.
            ui2 = uip.tile([128, 2, B], BF16, tag="ui2")
            mlp_layer(1, lambda k: ui1[:, k, :], ui2,
                      lambda k: wu23_s[0][:, k, :], bu_s, B,
                      order=[(m, k) for k in range(4) for m in range(2)])
            ui3 = uip.tile([128, 1, B], F32, tag="ui3")
            mlp_layer(2, lambda k: ui2[:, k, :], ui3,
                      lambda k: wu23_s[1][:, k, :], bu_s, B,
                      order=[(0, k) for k in range(2)])

            # chunk 0 expert MLP layer 1 keeps the PE fed while the ui3
            # act/transpose chain completes
            h1_c0 = actp.tile([128, 4, chunks[0][1]], BF16, tag="h1",
                              name="h1c0")
            mlp_layer(0, lambda k: xcs[0][:, k, :], h1_c0,
                      lambda k: w_s[0][:, k, :], b_s, chunks[0][1])

            # transpose ui3 [128d x B] -> token-major [128b x BC x 128d]:
            # 4 PE transposes into one PSUM bank, one wide DVE copy out.
            tps = mmps.tile([128, 512], F32, tag="mm", name="tps")
            for c in range(BC):
                nc.tensor.transpose(tps[:, c * 128:(c + 1) * 128],
                                    ui3[:, 0, c * 128:(c + 1) * 128], ident[:])
            ui3_tok = uip.tile([128, BC, 128], BF16, tag="ui3tok")
            nc.vector.tensor_copy(ui3_tok[:], tps[:])

            # ---- token chunks ----
            for ci, (s0, n) in enumerate(chunks):
                h3 = chunk_mlp(ci, n, h1=h1_c0 if ci == 0 else None)

                cs = chunk_cs[ci]
                # one-hot[b, t] = (b == bids[t]) for the b-chunks present
                oh = actp.tile([128, BC, n], BF16, tag="oh")
                for c in cs:
                    nc.vector.tensor_scalar(
                        out=oh[:, c, :], in0=bids_s[:, s0:s0 + n],
                        scalar1=iota_s[:, c:c + 1], scalar2=None,
                        op0=mybir.AluOpType.is_equal)
                # gathered ui columns: uig = ui3_tok.T @ oh  (exact selection)
                psg = mmps.tile([128, 512], F32, tag="mm")
                for j, c in enumerate(cs):
                    nc.tensor.matmul(psg[:, :n], ui3_tok[:, c, :], oh[:, c, :],
                                     start=(j == 0), stop=(j == len(cs) - 1))
                # pred = ones.T @ (h3 * uig)   (partition reduce over d=128)
                prod = actp.tile([128, n], BF16, tag="prod")
                nc.vector.tensor_tensor(out=prod[:], in0=h3[:, 0, :],
                                        in1=psg[:, :n],
                                        op=mybir.AluOpType.mult)
                psd = dotps.tile([1, 512], F32, tag="dot")
                nc.tensor.matmul(psd[:, :n], ones[:], prod[:],
                                 start=True, stop=True)
                pc = actp.tile([1, n], F32, tag="predc")
                nc.vector.tensor_copy(pc[:], psd[:, :n])
                nc.sync.dma_start(pred_d[:, s0:s0 + n], pc[:])

    nc.compile()
    return nc


def _prepare(u_emb, i_emb, a_emb, o_emb, s):
    """Host-side sharding: route tokens to cores by relation id."""
    s_flat = np.asarray(s).reshape(-1).astype(np.int64)
    n_tokens = s_flat.shape[0]
    X = np.concatenate(
        [np.asarray(a_emb, dtype=np.float32).reshape(n_tokens, IN1),
         np.asarray(o_emb, dtype=np.float32).reshape(n_tokens, IN1)],
        axis=1).astype(BF16_NP)
    uiT = np.ascontiguousarray(
        np.concatenate([np.asarray(u_emb, dtype=np.float32),
                        np.asarray(i_emb, dtype=np.float32)],
                       axis=1).astype(BF16_NP).T)

    idx = [np.flatnonzero(s_flat == e) for e in range(R)]
    tp = max(256, -(-max(max(len(ix) for ix in idx), 1) // 4) * 4)

    iota4 = np.ascontiguousarray(
        (np.arange(128, dtype=np.float32)[:, None] +
         128.0 * np.arange(4, dtype=np.float32)[None, :]))

    in_maps = []
    chunks = _chunks(tp)
    chunk_cs = [set() for _ in chunks]
    for e in range(R):
        # sort tokens by batch row within the expert: narrows per-chunk b-range
        order = np.argsort(idx[e] // N_TOK, kind="stable")
        idx[e] = idx[e][order]
        ix = idx[e]
        pad = np.full(tp, n_tokens - 1, dtype=np.int64)
        pad[:len(ix)] = ix
        xT = np.ascontiguousarray(X[pad].T)
        b_of_tok = pad // N_TOK
        bids = np.ascontiguousarray(np.broadcast_to(
            b_of_tok.astype(np.float16)[None, :], (128, tp)))
        cg = b_of_tok // 128
        for ci, (s0, n) in enumerate(chunks):
            for c in np.unique(cg[s0:s0 + n]):
                chunk_cs[ci].add(int(c))
        in_maps.append({"xT": xT, "uiT": uiT, "bids": bids, "iota4": iota4})
    chunk_cs = [sorted(cset) for cset in chunk_cs]
    return in_maps, idx, tp, chunk_cs


def _pack_bias(bs):
    """Stack per-layer bias vectors into a [128, n_cols] f32 pack."""
    cols = []
    for b in bs:
        cols.append(np.asarray(b, dtype=np.float32).reshape(-1, 128).T)
    return np.ascontiguousarray(np.concatenate(cols, axis=1))


def kernel(u_emb, i_emb, a_emb, o_emb, s,
           W1, b1, W2, b2, W3, b3,
           Wu1, bu1, Wu2, bu2, Wu3, bu3):
    in_maps, idx, tp, chunk_cs = _prepare(u_emb, i_emb, a_emb, o_emb, s)
    ws = {"W1": W1, "W2": W2, "W3": W3, "Wu1": Wu1, "Wu2": Wu2, "Wu3": Wu3}
    bupack = _pack_bias([bu1, bu2, bu3])
    for e in range(R):
        m = in_maps[e]
        for li in range(3):
            m[f"w{li+1}"] = np.ascontiguousarray(
                np.asarray(ws[f"W{li+1}"][e], dtype=np.float32).astype(BF16_NP))
            m[f"wu{li+1}"] = np.ascontiguousarray(
                np.asarray(ws[f"Wu{li+1}"], dtype=np.float32).astype(BF16_NP))
        m["bpack"] = _pack_bias([b1[e], b2[e], b3[e]])
        m["bupack"] = bupack

    nc = _build_kernel(tp, chunk_cs)
    res = run_bass_kernel_spmd(nc, in_maps, core_ids=list(range(N_CORES)))

    s_arr = np.asarray(s)
    out = np.zeros(s_arr.size, dtype=np.float32)
    for e in range(R):
        pred = res.results[e]["pred"].reshape(-1)
        out[idx[e]] = pred[:len(idx[e])]
    return out.reshape(s_arr.shape)
